# revision 14
# baseline (speedup 1.0000x reference)
"""Channel-attention block (GroupNorm -> qkv 1x1 -> attention over C -> proj + residual)
on 8 Trainium2 NeuronCores.  Batch 16 is sharded 2 samples/core; each core runs an
identical Bass/Tile program on its 2 samples.

Layouts (per sample, S = H*W = 1024 spatial, C = 768 channels):
  x, xn, v, o3 : [C, S]   (channel on partitions)
  qT, kT       : [S, 2C] stored as qk[st][128, 1536] (q cols 0:768, k cols 768:1536)
  attT (=E)    : [C_j, C_i]  (softmax axis j on partitions; sum over j via ones-matmul)
All matmuls run in float32r (fp32 storage, full-rate PE mode).
"""
import sys

for _p in ("/opt/trn_rl_repo",):
    if _p not in sys.path:
        sys.path.append(_p)

import numpy as np
import concourse.bass as bass
import concourse.mybir as mybir
import concourse.tile as tile
from concourse.bass_utils import run_bass_kernel_spmd

# ---------------------------------------------------------------------------
# Workaround for this container's walrus build: the TileContext exit drain
# carries one sync-wait per outstanding semaphore, but CoreV3 setupSyncWait
# accepts only a single wait on CTRL-class instructions.  Split the waits
# across individual SP nop instructions (SP program order preserves the
# semantics), then drain with no attached waits.
from concourse.vector_clock import ScopedClock


def _patched_drain_and_barrier(self, tick_clock, wait_clock):
    nc = self.nc
    probe = nc.sync.nop()
    wait_clock.add_sem_waits(probe.ins, ScopedClock({None: tick_clock.global_clock}))
    waits = list(probe.ins.sync_info.on_wait) if probe.ins.sync_info else []
    if probe.ins.sync_info:
        probe.ins.sync_info.on_wait = waits[:1]
    for w in waits[1:]:
        n = nc.sync.nop()
        n.ins.sync_info = mybir.SyncInfo(on_wait=[w], on_update=[])
    nc.sync.drain()
    nc.all_engine_barrier()
    assert self.sems is not None
    popped = nc._tile_sem_poison_stack.pop()
    assert popped is self._sem_poison
    nc.clear_and_free_semaphores(list(self.sems.allocated().values()))
    nc.all_engine_barrier()


tile.TileContext._drain_and_barrier = _patched_drain_and_barrier

_split_ctr = [0]


def _split_multi_waits(nc, limit=1):
    """Same walrus quirk as above, for every instruction: move excess sync
    waits onto same-engine nop instructions placed immediately before."""
    for f in nc.m.functions:
        for blk in f.blocks:
            new = []
            changed = False
            for inst in blk.instructions:
                si = inst.sync_info
                if si is not None and si.on_wait and len(si.on_wait) > limit:
                    waits = list(si.on_wait)
                    for w in waits[:-limit]:
                        nop = mybir.InstNoOp(
                            name=f"waitsplit_{_split_ctr[0]}", ins=[], outs=[])
                        _split_ctr[0] += 1
                        nop.engine = inst.engine
                        nop.sync_info = mybir.SyncInfo(on_wait=[w], on_update=[])
                        new.append(nop)
                    si.on_wait = waits[-limit:]
                    changed = True
                new.append(inst)
            if changed:
                blk.instructions = new
# ---------------------------------------------------------------------------

NCORES = 8
B, C, S = 16, 768, 1024
G = 32                      # groups
PER = B // NCORES           # samples per core
CT = C // 128               # 6 channel tiles
ST = S // 128               # 8 spatial tiles
EPS = 1e-5
F32 = mybir.dt.float32
F32R = mybir.dt.float32r
ACT_FN = mybir.ActivationFunctionType
ALU = mybir.AluOpType


def _r(ap):
    return ap.bitcast(F32R)


import os

_STAGE = os.environ.get("K_STAGE", "F")  # debug: cut the program after a stage


def _build(with_qkv_bias: bool):
    nc = bass.Bass()
    xs = nc.dram_tensor("xs", [PER, C, S], F32, kind="ExternalInput")
    wqkvT = nc.dram_tensor("wqkvT", [C, 3 * C], F32R, kind="ExternalInput")
    woutT = nc.dram_tensor("woutT", [C, C], F32R, kind="ExternalInput")
    bqkv_col = nc.dram_tensor("bqkv_col", [3 * C, 1], F32, kind="ExternalInput")
    bout_col = nc.dram_tensor("bout_col", [C, 1], F32, kind="ExternalInput")
    gamma_col = nc.dram_tensor("gamma_col", [C, 1], F32, kind="ExternalInput")
    beta_col = nc.dram_tensor("beta_col", [C, 1], F32, kind="ExternalInput")
    gmask = nc.dram_tensor("gmask", [C, G], F32R, kind="ExternalInput")      # 1/(C/G) entries
    gmaskT = nc.dram_tensor("gmaskT", [G, C], F32R, kind="ExternalInput")
    brow = nc.dram_tensor("brow", [1, 3 * C], F32R, kind="ExternalInput")
    ones_dram = nc.dram_tensor("ones_dram", [128, 4], F32R, kind="ExternalInput")
    out = nc.dram_tensor("out", [PER, C, S], F32, kind="ExternalOutput")

    with tile.TileContext(nc) as tc, \
         nc.allow_low_precision("fp32r compute by design"), \
         tc.tile_pool(name="big", bufs=1) as big, \
         tc.tile_pool(name="wpool", bufs=2) as wpool, \
         tc.tile_pool(name="small", bufs=1) as small, \
         tc.tile_pool(name="work", bufs=3) as work, \
         tc.tile_pool(name="psum", bufs=2, space="PSUM") as psum:

        # ---- constants (loaded once) ----
        gmask_sb, gamma_sb, beta_sb, bv_sb, bo_sb = [], [], [], [], []
        for ct in range(CT):
            t = small.tile([128, G], F32R, name=f"gmask{ct}", tag=f"gmask{ct}")
            nc.sync.dma_start(out=t, in_=gmask[ct * 128:(ct + 1) * 128, :])
            gmask_sb.append(t)
            g = small.tile([128, 1], F32, name=f"gam{ct}", tag=f"gam{ct}")
            nc.sync.dma_start(out=g, in_=gamma_col[ct * 128:(ct + 1) * 128, :])
            gamma_sb.append(g)
            bt = small.tile([128, 1], F32, name=f"bet{ct}", tag=f"bet{ct}")
            nc.sync.dma_start(out=bt, in_=beta_col[ct * 128:(ct + 1) * 128, :])
            beta_sb.append(bt)
            bv = small.tile([128, 1], F32, name=f"bv{ct}", tag=f"bv{ct}")
            nc.sync.dma_start(out=bv, in_=bqkv_col[2 * C + ct * 128: 2 * C + (ct + 1) * 128, :])
            bv_sb.append(bv)
            bo = small.tile([128, 1], F32, name=f"bo{ct}", tag=f"bo{ct}")
            nc.sync.dma_start(out=bo, in_=bout_col[ct * 128:(ct + 1) * 128, :])
            bo_sb.append(bo)
        gmT_sb = small.tile([G, C], F32R, name="gmT", tag="gmT")
        nc.sync.dma_start(out=gmT_sb, in_=gmaskT[:, :])
        eps_sb = small.tile([G, 1], F32, name="eps_sb", tag="eps")
        nc.vector.memset(eps_sb, EPS)
        ones_col = small.tile([128, 4], F32R, name="ones_col", tag="ones_col")
        nc.sync.dma_start(out=ones_col, in_=ones_dram[:, :])
        if with_qkv_bias:
            brow_sb = small.tile([1, 3 * C], F32R, name="brow_sb", tag="brow")
            nc.sync.dma_start(out=brow_sb, in_=brow[:, :])
            ones1 = small.tile([1, 128], F32R, name="ones1", tag="ones1")
            nc.sync.dma_start(out=ones1, in_=ones_dram[:, 0:1].rearrange("p one -> one p"))


        def _dump(tiles, ncols, s):
            for i, t in enumerate(tiles[:CT]):
                nc.sync.dma_start(
                    out=out[s, i * 128:(i + 1) * 128, 0:ncols].bitcast(t.dtype),
                    in_=t[:, 0:ncols])
        for s in range(PER):
            # ================= Stage A: GroupNorm =================
            x_sb = []
            for ct in range(CT):
                xt = big.tile([128, S], F32, name=f"x{ct}", tag=f"x{ct}")
                nc.sync.dma_start(out=xt, in_=xs[s, ct * 128:(ct + 1) * 128, :])
                x_sb.append(xt)

            st2 = []  # per-channel [mean, E[x^2]]
            for ct in range(CT):
                stats = work.tile([128, 2, 6], F32, name="stats", tag="stats")
                nc.vector.bn_stats(out=stats[:, 0, :], in_=x_sb[ct][:, 0:512])
                nc.vector.bn_stats(out=stats[:, 1, :], in_=x_sb[ct][:, 512:1024])
                mv = work.tile([128, 2], F32, name="mv", tag="mv")
                nc.vector.bn_aggr(out=mv, in_=stats)
                s2 = work.tile([128, 2], F32R, name="s2", tag=f"s2_{ct}", bufs=2)
                nc.vector.tensor_copy(s2[:, 0:1], mv[:, 0:1])
                # s2[:,1] = mean^2 + var = E[x^2]
                nc.vector.scalar_tensor_tensor(
                    out=s2[:, 1:2], in0=mv[:, 0:1], scalar=mv[:, 0:1], in1=mv[:, 1:2],
                    op0=ALU.mult, op1=ALU.add)
                st2.append(s2)

            pg = psum.tile([G, 2], F32, name="pg", tag="pz")
            for ct in range(CT):
                nc.tensor.matmul(pg, _r(gmask_sb[ct]), st2[ct],
                                 start=(ct == 0), stop=(ct == CT - 1))
            # group stats -> grp2[0:G] = [rstd_g, m_g]
            grp2 = work.tile([G, 2], F32R, name="grp2", tag="grp2")
            pgs = work.tile([G, 2], F32, name="pgs", tag="pgs")
            nc.vector.tensor_copy(pgs, pg)
            m2 = work.tile([G, 1], F32, name="m2", tag="m2")
            nc.vector.tensor_mul(m2, pgs[:, 0:1], pgs[:, 0:1])
            var_g = work.tile([G, 1], F32, name="var_g", tag="var_g")
            nc.vector.tensor_sub(var_g, pgs[:, 1:2], m2)
            sd_g = work.tile([G, 1], F32, name="sd_g", tag="sd_g")
            nc.scalar.activation(out=sd_g, in_=var_g, func=ACT_FN.Sqrt, bias=eps_sb)
            nc.vector.reciprocal(out=grp2[:, 0:1], in_=sd_g)
            nc.vector.tensor_copy(grp2[:, 1:2], pgs[:, 0:1])

            xn_sb = []
            for ct in range(CT):
                pcb = psum.tile([128, 2], F32, name="pcb", tag="pz")
                nc.tensor.matmul(pcb, _r(gmT_sb[:, ct * 128:(ct + 1) * 128]), grp2,
                                 start=True, stop=True)
                scale_c = work.tile([128, 1], F32, name="scale_c", tag=f"scale{ct}", bufs=2)
                nc.vector.tensor_mul(scale_c, gamma_sb[ct], pcb[:, 0:1])
                # shift = beta - m*scale:  stt gives (m*scale - beta); negate after
                shift_c = work.tile([128, 1], F32, name="shift_c", tag=f"shift{ct}", bufs=2)
                nc.vector.scalar_tensor_tensor(
                    out=shift_c, in0=pcb[:, 1:2], scalar=scale_c, in1=beta_sb[ct],
                    op0=ALU.mult, op1=ALU.subtract)
                nc.scalar.activation(out=shift_c, in_=shift_c, func=ACT_FN.Copy,
                                     bias=0.0, scale=-1.0)
                xt = big.tile([128, S], F32R, name=f"xn{ct}", tag=f"xo{ct}")
                nc.scalar.activation(out=xt, in_=x_sb[ct], func=ACT_FN.Identity,
                                     bias=shift_c, scale=scale_c)
                xn_sb.append(xt)

            if _STAGE == "A":
                _dump(xn_sb, S, s)
                continue

            # ================= Stage B: qT / kT =================
            qk_sb = []
            for st in range(ST):
                t = big.tile([128, 2 * C], F32R, name=f"qk{st}", tag=f"qk{st}")
                qk_sb.append(t)
            for oc in range(3):  # 512-wide chunks of the 1536 q|k output cols
                wch = []
                for ct in range(CT):
                    w = wpool.tile([128, C], F32R, name=f"w{ct}", tag=f"w{ct}")
                    nc.sync.dma_start(
                        out=w[:, 0:512],
                        in_=wqkvT[ct * 128:(ct + 1) * 128, oc * 512:(oc + 1) * 512])
                    wch.append(w)
                for st in range(ST):
                    pq = psum.tile([128, 512], F32, name="pq", tag="mm")
                    for ct in range(CT):
                        nc.tensor.matmul(
                            pq, xn_sb[ct][:, st * 128:(st + 1) * 128],
                            _r(wch[ct][:, 0:512]),
                            start=(ct == 0),
                            stop=(ct == CT - 1 and not with_qkv_bias))
                    if with_qkv_bias:
                        nc.tensor.matmul(
                            pq, _r(ones1),
                            _r(brow_sb[:, oc * 512:(oc + 1) * 512]),
                            start=False, stop=True)
                    nc.vector.tensor_copy(qk_sb[st][:, oc * 512:(oc + 1) * 512], pq)

            if _STAGE == "B":
                _dump(qk_sb, S, s)
                continue

            # ================= Stage C: v =================
            v_sb = []
            for ct in range(CT):
                t = big.tile([128, S], F32R, name=f"v{ct}", tag=f"v{ct}")
                v_sb.append(t)
            for ot in range(CT):
                wv_t = []
                for ct in range(CT):
                    w = wpool.tile([128, 128], F32R, name=f"wv{ct}", tag=f"wv{ct}")
                    nc.sync.dma_start(
                        out=w,
                        in_=wqkvT[ct * 128:(ct + 1) * 128,
                                  2 * C + ot * 128:2 * C + (ot + 1) * 128])
                    wv_t.append(w)
                for sc in range(2):
                    pv = psum.tile([128, 512], F32, name="pv", tag="mm")
                    for ct in range(CT):
                        nc.tensor.matmul(
                            pv, _r(wv_t[ct]), xn_sb[ct][:, sc * 512:(sc + 1) * 512],
                            start=(ct == 0), stop=(ct == CT - 1))
                    nc.scalar.activation(
                        out=v_sb[ot][:, sc * 512:(sc + 1) * 512], in_=pv,
                        func=ACT_FN.Identity, bias=bv_sb[ot])

            if _STAGE == "C":
                _dump(v_sb, S, s)
                continue

            # ================= Stage D: attT = exp((kT.T qT) / 32) =================
            E_sb = []
            for jt in range(CT):
                t = big.tile([128, C], F32R, name=f"E{jt}", tag=f"E{jt}")
                E_sb.append(t)
            for jt in range(CT):
                for i0, iw in ((0, 512), (512, 256)):
                    pa = psum.tile([128, 512], F32, name="pa", tag="mm")
                    for st in range(ST):
                        nc.tensor.matmul(
                            pa[:, 0:iw],
                            qk_sb[st][:, C + jt * 128:C + (jt + 1) * 128],
                            qk_sb[st][:, i0:i0 + iw],
                            start=(st == 0), stop=(st == ST - 1))
                    nc.scalar.activation(
                        out=E_sb[jt][:, i0:i0 + iw], in_=pa[:, 0:iw],
                        func=ACT_FN.Exp, scale=float(S) ** -0.5)

            if _STAGE == "D":
                _dump(E_sb, C, s)
                continue

            # ================= Stage E: o3 = (E.T @ V) / Z =================
            o3_sb = []
            for it in range(CT):
                t = big.tile([128, S], F32R, name=f"o3_{it}", tag=f"xo{it}")
                o3_sb.append(t)
            for it in range(CT):
                pd0 = psum.tile([128, 512], F32, name="pd0", tag="pd0")
                pd1 = psum.tile([128, 512], F32, name="pd1", tag="pd1")
                pz = psum.tile([128, 4], F32, name="pzt", tag="pz")
                for jt in range(CT):
                    lhs = E_sb[jt][:, it * 128:(it + 1) * 128]
                    nc.tensor.matmul(pd0, lhs, v_sb[jt][:, 0:512],
                                     start=(jt == 0), stop=(jt == CT - 1))
                    nc.tensor.matmul(pd1, lhs, v_sb[jt][:, 512:1024],
                                     start=(jt == 0), stop=(jt == CT - 1))
                    nc.tensor.matmul(pz, lhs, ones_col,
                                     start=(jt == 0), stop=(jt == CT - 1))
                rz = work.tile([128, 1], F32, name="rz", tag="rz", bufs=2)
                nc.vector.reciprocal(out=rz, in_=pz[:, 0:1])
                nc.scalar.activation(out=o3_sb[it][:, 0:512], in_=pd0,
                                     func=ACT_FN.Copy, bias=0.0, scale=rz)
                nc.scalar.activation(out=o3_sb[it][:, 512:1024], in_=pd1,
                                     func=ACT_FN.Copy, bias=0.0, scale=rz)

            if _STAGE == "E":
                _dump(o3_sb, S, s)
                continue

            # ================= Stage F: out = w_out @ o3 + b_out + x =================
            wo_sb = []
            for ct in range(CT):
                w = wpool.tile([128, C], F32R, name=f"w{ct}", tag=f"w{ct}")
                nc.sync.dma_start(out=w, in_=woutT[ct * 128:(ct + 1) * 128, :])
                wo_sb.append(w)
            for ot in range(CT):
                for sc in range(2):
                    pf = psum.tile([128, 512], F32, name="pf", tag="mm")
                    for ct in range(CT):
                        nc.tensor.matmul(
                            pf, _r(wo_sb[ct][:, ot * 128:(ot + 1) * 128]),
                            o3_sb[ct][:, sc * 512:(sc + 1) * 512],
                            start=(ct == 0), stop=(ct == CT - 1))
                    ot_t = work.tile([128, 512], F32, name="ot_t", tag="ot_t", bufs=3)
                    nc.vector.scalar_tensor_tensor(
                        out=ot_t, in0=pf, scalar=bo_sb[ot],
                        in1=x_sb[ot][:, sc * 512:(sc + 1) * 512],
                        op0=ALU.add, op1=ALU.add)
                    nc.sync.dma_start(
                        out=out[s, ot * 128:(ot + 1) * 128, sc * 512:(sc + 1) * 512],
                        in_=ot_t)
    _split_multi_waits(nc)
    return nc


def _prepare_inputs(x, gn_gamma, gn_beta, w_qkv, b_qkv, w_out, b_out):
    x = np.asarray(x, dtype=np.float32)
    Bx, Cx, H, W = x.shape
    xs_all = np.ascontiguousarray(x.reshape(Bx, Cx, H * W))
    cg = Cx // G
    gmask = np.zeros((Cx, G), np.float32)
    gmask[np.arange(Cx), np.arange(Cx) // cg] = 1.0 / cg
    gmaskT = np.zeros((G, Cx), np.float32)
    gmaskT[np.arange(Cx) // cg, np.arange(Cx)] = 1.0
    shared = dict(
        wqkvT=np.ascontiguousarray(np.asarray(w_qkv, np.float32).T),
        woutT=np.ascontiguousarray(np.asarray(w_out, np.float32).T),
        bqkv_col=np.ascontiguousarray(np.asarray(b_qkv, np.float32).reshape(-1, 1)),
        bout_col=np.ascontiguousarray(np.asarray(b_out, np.float32).reshape(-1, 1)),
        gamma_col=np.ascontiguousarray(np.asarray(gn_gamma, np.float32).reshape(-1, 1)),
        beta_col=np.ascontiguousarray(np.asarray(gn_beta, np.float32).reshape(-1, 1)),
        gmask=gmask, gmaskT=gmaskT,
        ones_dram=np.ones((128, 4), np.float32),
        brow=np.ascontiguousarray(np.asarray(b_qkv, np.float32).reshape(1, -1)),
    )
    in_maps = [dict(xs=np.ascontiguousarray(xs_all[c * PER:(c + 1) * PER]), **shared)
               for c in range(NCORES)]
    with_qkv_bias = bool(np.any(np.asarray(b_qkv)[: 2 * Cx]))
    return in_maps, with_qkv_bias, (Bx, Cx, H, W)


def _run(inputs, **spmd_kwargs):
    in_maps, with_qkv_bias, shape = _prepare_inputs(**inputs)
    nc = _build(with_qkv_bias)
    res = run_bass_kernel_spmd(nc, in_maps, core_ids=list(range(NCORES)), **spmd_kwargs)
    Bx, Cx, H, W = shape
    out = np.concatenate([res.results[c]["out"] for c in range(NCORES)], axis=0)
    return out.reshape(Bx, Cx, H, W), res


def kernel(x, gn_gamma, gn_beta, w_qkv, b_qkv, w_out, b_out):
    out, _ = _run(dict(x=x, gn_gamma=gn_gamma, gn_beta=gn_beta, w_qkv=w_qkv,
                       b_qkv=b_qkv, w_out=w_out, b_out=b_out))
    return out
